# revision 20
# baseline (speedup 1.0000x reference)
"""Causal self-attention on 8 Trainium2 NeuronCores.

Sharding: 4 batches x 2 head-groups (8 heads each per core). Every core runs
the same SPMD program on its (batch, head-group) slice and emits a bf16
partial projection output [T, C]; the host sums the two head-group partials
per batch and adds b_proj while unsharding.

Per-core program (bf16 matmuls, fp32 accumulation, ~355-365us on HW):
  - phase 1: six x tiles prefetched ahead of the weight DMAs; x cast to bf16
    on ACT, PE-transposed in bf16; v computed token-major in 65-wide head
    blocks with a ones column so the AV matmul also emits the softmax
    denominator; q/k chunk GEMMs for head pair 0.
  - phase 2 (attention): per head pair m, per 512-query chunk: 2-head
    row-group-packed score matmuls -> per-128-key-block exp on ACT (no max
    subtraction; scores are O(1)) -> causal tri-mask on GpSimd -> 65-row AV
    accumulation. The q/k GEMMs for head pair m+1 ride in this stream every
    5th key block so the tensor engine stays dense (and HAM-warm).
  - normalization at each stage boundary: denominator rows striped onto the
    32-partition offsets, one batched PE transpose, one wide DVE reciprocal,
    PE transpose back, K=1 broadcast matmuls scale the fp16 attention output
    in place. (Interleaving these serial DVE->PE chains into the attention
    stream measurably regresses; keep them at boundaries.)
  - phase 3: the fp16 output projection is interleaved with the last head
    pair's normalization per query chunk (each normalized chunk unlocks 4
    projection tiles), its PSUM rotating over three banks so the matmul
    chains never wait on an ACT drain; ACT converts PSUM->bf16, DMA out.
    A zero-bias build variant (the harness case) skips the v-bias matmuls;
    kernel() dispatches on the actual bias values.
"""

import sys

for _p in ("/opt/trn_rl_repo", "/root/.axon_site/_ro/trn_rl_repo"):
    if _p not in sys.path:
        sys.path.append(_p)

import numpy as np

import concourse.bass as bass
import concourse.mybir as mybir
import concourse.tile as tile
from concourse.bass import ts
from concourse.bass_utils import run_bass_kernel_spmd
from concourse.masks import make_identity, make_upper_triangular
from concourse.vector_clock import ScopedClock

F32 = mybir.dt.float32
F16 = mybir.dt.float16
BF16 = mybir.dt.bfloat16
AF = mybir.ActivationFunctionType

B, T, C, H, DH = 4, 2048, 1024, 16, 64
G = 2              # head-groups
HG = H // G        # heads per core (8)
CG = HG * DH       # channels per core (512)
NT = T // 128      # 16 token tiles
NQC = T // 512     # 4 query chunks
NCK = CG // 128    # 4 channel chunks / head pairs
SCALE = DH ** -0.5

MAX_WAITS = 1      # this walrus build allows one sync wait per instruction


class TC(tile.TileContext):
    """TileContext whose tail drain splits sem waits across nops."""

    def _drain_and_barrier(self, tick_clock, wait_clock):
        probe = self.nc.sync.nop()
        wait_clock.add_sem_waits(
            probe.ins, ScopedClock({None: tick_clock.global_clock})
        )
        si = probe.ins.sync_info
        waits = list(si.on_wait) if si is not None else []
        if len(waits) > MAX_WAITS:
            si.on_wait[:] = waits[:MAX_WAITS]
            for i in range(MAX_WAITS, len(waits), MAX_WAITS):
                n = self.nc.sync.nop()
                nsi = n.ins.sync_info
                if nsi is None:
                    n.ins.sync_info = mybir.SyncInfo(
                        on_wait=list(waits[i : i + MAX_WAITS]), on_update=[]
                    )
                else:
                    nsi.on_wait.extend(waits[i : i + MAX_WAITS])
        self.nc.sync.drain()
        self.nc.all_engine_barrier()
        assert self.sems is not None
        popped = self.nc._tile_sem_poison_stack.pop()
        assert popped is self._sem_poison
        self.nc.clear_and_free_semaphores(list(self.sems.allocated().values()))
        self.nc.all_engine_barrier()


def split_excess_waits(nc, max_waits=MAX_WAITS):
    uid = 0
    for f in nc.m.functions:
        for bb in f.blocks:
            insts = list(bb.instructions)
            out = []
            changed = False
            for inst in insts:
                si = inst.sync_info
                if si is not None and len(si.on_wait) > max_waits:
                    waits = list(si.on_wait)
                    extra = waits[max_waits:]
                    for gi in range(0, len(extra), max_waits):
                        uid += 1
                        out.append(
                            mybir.InstNoOp(
                                name=f"I-wsplit-{uid}",
                                engine=inst.engine,
                                sync_info=mybir.SyncInfo(
                                    on_wait=list(extra[gi : gi + max_waits]),
                                    on_update=[],
                                ),
                            )
                        )
                    inst.sync_info = mybir.SyncInfo(
                        on_wait=waits[:max_waits], on_update=list(si.on_update)
                    )
                    changed = True
                out.append(inst)
            if changed:
                bb.instructions[:] = out


def build(for_sim=False, zero_bias=False):
    nc = bass.Bass()
    x_d = nc.declare_dram_parameter("x", [T, C], F32, isOutput=False)
    wqkv_d = nc.declare_dram_parameter("wqkv", [C, 3 * CG], F32, isOutput=False)
    bqkv_d = nc.declare_dram_parameter("bqkv", [3 * CG], F32, isOutput=False)
    wp_d = nc.declare_dram_parameter("wp", [CG, C], F32, isOutput=False)
    yp_d = nc.declare_dram_parameter("yp", [T, C], BF16, isOutput=True)

    from contextlib import ExitStack

    tc_cls = tile.TileContext if for_sim else TC
    with tc_cls(nc) as tc, ExitStack() as phases:
        with (
            tc.tile_pool(name="persist", bufs=1) as persist,
            tc.tile_pool(name="attn", bufs=4) as attn,
        ):
            # ---- constants ----
            tri = persist.tile([128, 128], BF16, tag="tri")
            make_upper_triangular(nc, tri[:], val=1.0, diag=True)
            identf = persist.tile([128, 128], F32, tag="identf")
            make_identity(nc, identf[:])
            identb = persist.tile([128, 128], BF16, tag="identb")
            nc.vector.tensor_copy(identb[:], identf[:])
            bqs = persist.tile([128, 8], F32, tag="bqs")  # q,k bias chunks
            bvr = persist.tile([1, CG], F32, tag="bvr")  # v bias row
            bvb_row = persist.tile([1, CG], BF16, tag="bvb_row")
            ones128b = persist.tile([1, 128], BF16, tag="ones128b")
            nc.vector.memset(ones128b[:], 1.0)
            ones64h = persist.tile([1, 64], F16, tag="ones64h")
            nc.vector.memset(ones64h[:], 1.0)
            # warm the ACT exp table before the attention phase needs it
            warm = persist.tile([128, 8], F32, tag="warm")
            nc.vector.memset(warm[:], 0.0)
            nc.scalar.activation(warm[:], warm[:], AF.Exp)

            # ---- persistent activations ----
            xtp = phases.enter_context(tc.tile_pool(name="xtp", bufs=1))
            stage = phases.enter_context(tc.tile_pool(name="stage", bufs=2))
            outp = phases.enter_context(tc.tile_pool(name="outp", bufs=2))
            xTall = xtp.tile([128, 8 * T], BF16, tag="xTall")
            xT3 = xTall[:].rearrange("p (a t) -> p a t", t=T)
            qT = [persist.tile([128, T], BF16, tag=f"qT{c}", name=f"qT{c}") for c in range(NCK)]
            kT = [persist.tile([128, T], BF16, tag=f"kT{c}", name=f"kT{c}") for c in range(NCK)]
            vA = [persist.tile([128, HG * 65], BF16, tag=f"vA{t}", name=f"vA{t}") for t in range(NT)]
            OT = [persist.tile([128, T], F16, tag=f"OT{c}", name=f"OT{c}") for c in range(NCK)]
            wqkb = [persist.tile([128, C], BF16, tag=f"wqkb{c}", name=f"wqkb{c}") for c in range(8)]
            wvb = persist.tile([128, 8 * CG], BF16, tag="wvb")
            wpb = [persist.tile([128, C], F16, tag=f"wpb{c}", name=f"wpb{c}") for c in range(NCK)]
            # l rows per head: row 32*qc holds that query-chunk's denominators
            lq = [
                persist.tile([97, 512], F32, tag=f"lq{h}", name=f"lq{h}")
                for h in range(2)
            ]  # reused per m
            rback = persist.tile([1, 2 * T], F16, tag="rback")  # [qA | qB], per m
            # reciprocal lands striped: col 97*(4h+i) + 32*qc holds r chunk
            rS = persist.tile([128, 8 * 97], F32, tag="rS")
            for h in range(2):
                nc.vector.memset(lq[h][:], 1.0)  # untouched partitions stay finite

            # ---------- phase 1: load/transpose x, weights, v ----------
            with tc.tile_pool(name="p1", bufs=1, space="PSUM") as p1:
                # prefetch the first x tiles so the PE transposes start
                # immediately; weight DMAs trail behind them on the queue
                xf_pre = []
                for tt in range(6):
                    xf = stage.tile([128, C], F32, tag="xf", bufs=5, name=f"xfp{tt}")
                    nc.sync.dma_start(xf[:], x_d[ts(tt, 128), :])
                    xf_pre.append(xf)

                # bias loads ride behind the x prefetch on the DMA queue
                for j in range(8):
                    nc.sync.dma_start(bqs[:, j : j + 1], bqkv_d[ts(j, 128)])
                nc.sync.dma_start(bvr[:], bqkv_d[2 * CG : 3 * CG])
                nc.vector.tensor_copy(bvb_row[:], bvr[:])

                wv_src = wqkv_d[:, 2 * CG : 3 * CG].rearrange(
                    "(a p) c -> p a c", p=128
                )
                for q4 in range(4):
                    wvf = stage.tile([128, 1024], F32, tag="wvf", bufs=2)
                    nc.sync.dma_start(
                        wvf[:].rearrange("p (a c) -> p a c", a=2),
                        wv_src[:, 2 * q4 : 2 * q4 + 2, :],
                    )
                    nc.scalar.copy(wvb[:, ts(q4, 1024)], wvf[:])

                def load_wqk(co):
                    wf = stage.tile([128, C], F32, tag="wf", bufs=2)
                    nc.sync.dma_start(
                        wf[:].rearrange("p (a c) -> p a c", a=8),
                        wqkv_d[:, ts(co, 128)].rearrange("(a p) c -> p a c", p=128),
                    )
                    nc.scalar.copy(wqkb[co][:], wf[:])

                for co in (0, 4):
                    load_wqk(co)

                def emit_v(tt, pool=None, tag="psc", bufs=3):
                    ps = (pool or p1).tile([128, CG], F32, tag=tag, bufs=bufs, name="psv")
                    for a in range(8):
                        nc.tensor.matmul(
                            ps[:],
                            xT3[:, a, ts(tt, 128)],
                            wvb[:, ts(a, CG)],
                            start=(a == 0),
                            stop=(a == 7) if zero_bias else False,
                        )
                    if not zero_bias:
                        nc.tensor.matmul(  # += broadcast v bias (K=1 ones row)
                            ps[:], ones128b[:], bvb_row[:], start=False, stop=True
                        )
                    v3 = vA[tt][:].rearrange("p (h c) -> p h c", c=65)
                    nc.vector.tensor_copy(
                        v3[:, :, 0:DH],
                        ps[:].rearrange("p (h c) -> p h c", c=DH),
                    )
                    nc.vector.memset(v3[:, :, DH : DH + 1], 1.0)

                def emit_qk_group(co, tc4, pool, tag, bufs):
                    dest = qT[co] if co < NCK else kT[co - NCK]
                    psb = pool.tile([128, 512], F32, tag=tag, bufs=bufs)
                    for a in range(8):
                        nc.tensor.matmul(
                            psb[:],
                            wqkb[co][:, ts(a, 128)],
                            xT3[:, a, ts(tc4, 512)],
                            start=(a == 0),
                            stop=(a == 7),
                        )
                    nc.vector.tensor_scalar_add(
                        dest[:, ts(tc4, 512)], psb[:], bqs[:, co : co + 1]
                    )

                # x: stream fp32 tiles, PE-transpose (fp32 transpose mode),
                # cast bf16 on the PSUM copy-out; every 4 tiles, fold in the
                # v-compute and the m=0 q/k chunk GEMMs for that token range.
                for tt in range(NT):
                    if tt < 6:
                        xf = xf_pre[tt]
                    else:
                        xf = stage.tile([128, C], F32, tag="xf", bufs=5)
                        nc.sync.dma_start(xf[:], x_d[ts(tt, 128), :])
                    xb = stage.tile([128, C], BF16, tag="xb", bufs=3)
                    if tt % 2 == 0:
                        nc.scalar.copy(xb[:], xf[:])
                    else:
                        nc.vector.tensor_copy(xb[:], xf[:])
                    for g4 in range(2):
                        pt4 = p1.tile([128, 512], BF16, tag="pta", bufs=3, name="pt4")
                        for j in range(4):
                            a = 4 * g4 + j
                            nc.tensor.transpose(
                                pt4[:, ts(j, 128)], xb[:, ts(a, 128)], identb[:]
                            )
                        nc.vector.tensor_copy(
                            xT3[:, 4 * g4 : 4 * g4 + 4, ts(tt, 128)],
                            pt4[:].rearrange("p (j c) -> p j c", c=128),
                        )
                    if tt % 4 == 3:
                        tc4 = tt // 4
                        for tt2 in range(4 * tc4, 4 * tc4 + 4):
                            emit_v(tt2)
                        for co in (0, 4):
                            emit_qk_group(co, tc4, p1, "psb", 2)

                # remaining q/k weight chunks (cast during attention lead-in)
                for co in (1, 5, 2, 6, 3, 7):
                    load_wqk(co)

            # ---------- phase 2: attention with interleaved q/k GEMMs ----------
            # B-work queues: host_queue[m] holds the q/k chunk GEMMs for head
            # pair m+1, emitted into D(m)'s PE stream so the tensor engine
            # never idles while ACT paces exp. (Chunks 0/4 ran in phase 1.)
            host_queue = {m: [] for m in range(NCK)}
            for m in range(NCK - 1):
                for co in (m + 1, m + 5):
                    for tc4 in range(NQC):
                        host_queue[m].append((co, tc4))

            def emit_b_group(co, tc4, pool):
                dest = qT[co] if co < NCK else kT[co - NCK]
                psb = pool.tile([128, 512], F32, tag="psb", bufs=1)
                for a in range(8):
                    nc.tensor.matmul(
                        psb[:],
                        wqkb[co][:, ts(a, 128)],
                        xT3[:, a, ts(tc4, 512)],
                        start=(a == 0),
                        stop=(a == 7),
                    )
                nc.vector.tensor_scalar_add(
                    dest[:, ts(tc4, 512)], psb[:], bqs[:, co : co + 1]
                )

            def emit_wp_load(ck):
                wpf = stage.tile([128, C], F32, tag="wpf", bufs=1)
                nc.sync.dma_start(wpf[:], wp_d[ts(ck, 128), :])
                nc.vector.tensor_copy(wpb[ck][:], wpf[:])

            def make_norm_stages(m, wide):
                """Normalization for head pair m as deferred closures, so the
                PE ops spread into the next head pair's attention stream."""

                def nrm_tile():
                    if wide:  # score ring is free after the last exp
                        return pd.tile([128, 1024], F32, tag="pss", bufs=2, name="nrmwide")
                    return pd.tile([128, 512], F32, tag="nrm", bufs=1, name="nrmt")

                def stage1():
                    # transpose l rows (query-chunks batched on the
                    # 32-partition stripes) -> [q_sub, (h,i,qc)] columns
                    for h in range(2):
                        psT = nrm_tile()
                        for i in range(4):
                            nc.tensor.matmul(
                                psT[:, 97 * i : 97 * i + 97],
                                lq[h][:, ts(i, 128)],
                                identf[0:97, 0:97],
                                is_transpose=True,
                            )
                        # rS col layout: 97*(4h+i) + 32*qc
                        nc.vector.reciprocal(
                            rS[:, 388 * h : 388 * h + 388]
                            .rearrange("p (i c) -> p i c", c=97)[:, :, 0:97:32],
                            psT[:, 0:388]
                            .rearrange("p (i c) -> p i c", c=97)[:, :, 0:97:32],
                        )

                def stage2():
                    # transpose back -> r row [1, 2T] fp16 ([qA | qB]);
                    # rows stripe at 32*qc so single-row reads are legal
                    for h in range(2):
                        psTb = nrm_tile()
                        for i in range(4):
                            nc.tensor.matmul(
                                psTb[0:97, ts(i, 128)],
                                rS[:, 97 * (4 * h + i) : 97 * (4 * h + i) + 97],
                                identf[:],
                                is_transpose=True,
                            )
                        for qc in range(NQC):
                            nc.vector.tensor_copy(
                                rback[0:1, h * T + 512 * qc : h * T + 512 * qc + 512],
                                psTb[32 * qc : 32 * qc + 1, 0:512],
                            )

                def mul_stage(qcs):
                    def run():
                        for qc in qcs:
                            for h, hp in ((0, 0), (1, 64)):
                                R = nrm_tile()
                                nc.tensor.matmul(
                                    R[0:64, 0:512],
                                    ones64h[:],
                                    rback[0:1, h * T + 512 * qc : h * T + 512 * qc + 512],
                                    start=True,
                                    stop=True,
                                )
                                osl = OT[m][hp : hp + 64, ts(qc, 512)]
                                nc.vector.tensor_mul(osl, osl, R[0:64, 0:512])

                    return run

                return [stage1, stage2, mul_stage([0, 1]), mul_stage([2, 3])], mul_stage

            with tc.tile_pool(name="pd", bufs=1, space="PSUM") as pd:
                carry = []
                for m in range(NCK):  # head pair (2m, 2m+1)
                    pending = carry + [
                        (lambda c=co, t=tc4: emit_b_group(c, t, pd))
                        for co, tc4 in host_queue[m]
                    ]
                    kb_count = 0
                    for qc in range(NQC):
                        nkb = 4 * (qc + 1)
                        poA = pd.tile([65, 512], F32, tag="poA", bufs=1)
                        poB = pd.tile([65, 512], F32, tag="poB", bufs=1)
                        for kb in range(nkb):
                            j = kb - 4 * qc
                            c0 = 128 * j if j >= 0 else 0
                            qsl = slice(512 * qc + c0, 512 * (qc + 1))
                            ps = pd.tile([128, 1024], F32, tag="pss", bufs=2)
                            nc.tensor.matmul(
                                ps[:, c0:512],
                                kT[m][0:64, ts(kb, 128)],
                                qT[m][0:64, qsl],
                                start=True,
                                stop=True,
                            )
                            nc.tensor.matmul(
                                ps[:, 512 : 1024 - c0],
                                kT[m][64:128, ts(kb, 128)],
                                qT[m][64:128, qsl],
                                start=True,
                                stop=True,
                            )
                            pt = attn.tile([128, 1024], BF16, tag="pt")
                            nc.scalar.activation(
                                pt[:, c0 : 1024 - c0],
                                ps[:, c0 : 1024 - c0],
                                AF.Exp,
                                scale=SCALE,
                            )
                            if j >= 0:  # diagonal: causal mask both heads
                                for lo in (c0, 512):
                                    sl = slice(lo, lo + 128)
                                    nc.gpsimd.tensor_mul(pt[:, sl], pt[:, sl], tri[:])
                            nc.tensor.matmul(
                                poA[:, c0:512],
                                vA[kb][:, 65 * 2 * m : 65 * 2 * m + 65],
                                pt[:, c0:512],
                                start=(kb == 0),
                                stop=(kb == nkb - 1),
                            )
                            nc.tensor.matmul(
                                poB[:, c0:512],
                                vA[kb][:, 65 * (2 * m + 1) : 65 * (2 * m + 1) + 65],
                                pt[:, 512 : 1024 - c0],
                                start=(kb == 0),
                                stop=(kb == nkb - 1),
                            )
                            kb_count += 1
                            # interleave queued work (denser in D(0), which
                            # hosts the leftover phase-1 groups)
                            stride = 5 if m == 0 else 5
                            if pending and kb_count % stride == stride - 1:
                                pending.pop(0)()
                        # qc end: drain po accumulators
                        hp_slices = ((poA, 0, 0), (poB, 64, 1))
                        for poX, hp, li in hp_slices:
                            nc.vector.tensor_copy(
                                OT[m][hp : hp + 64, ts(qc, 512)], poX[0:64, :]
                            )
                            nc.vector.tensor_copy(
                                lq[li][32 * qc : 32 * qc + 1, :], poX[64:65, :]
                            )
                    # flush any unemitted interleave work for this stage
                    while pending:
                        pending.pop(0)()
                    if m == 2:
                        for ck in range(NCK):
                            emit_wp_load(ck)

                    # normalization inline at the stage boundary: its serial
                    # DVE->PE chains would stall the dense PE stream if
                    # interleaved into D(m+1) (measured +70us)
                    if m < NCK - 1:
                        for st in make_norm_stages(m, wide=False)[0]:
                            st()

                # last pair's normalization runs with the output projection
                # wrapped around it: F token tiles 0..7 only need OT[3]'s
                # first half, so they fill the PE while the second half's
                # reciprocal/broadcast chain serializes on DVE
                def emit_f(tt):
                    # rotate the projection PSUM over four distinct banks
                    # (score ring + the now-idle interleave/norm banks) so
                    # the matmul chains never wait on an ACT drain
                    ysb = outp.tile([128, C], BF16, tag="ysb", name="ysb")
                    for co2 in range(2):
                        k = (2 * tt + co2) % 3
                        if k < 2:
                            pw = pd.tile([128, 1024], F32, tag="pss", bufs=2, name="psy")
                            pslice = pw[:, 0:512]
                        else:
                            pw = pd.tile([128, 512], F32, tag="psb", bufs=1, name="psyb")
                            pslice = pw[:]
                        for ck in range(NCK):
                            nc.tensor.matmul(
                                pslice,
                                OT[ck][:, ts(tt, 128)],
                                wpb[ck][:, ts(co2, 512)],
                                start=(ck == 0),
                                stop=(ck == NCK - 1),
                            )
                        nc.scalar.copy(ysb[:, ts(co2, 512)], pslice)
                    nc.sync.dma_start(yp_d[ts(tt, 128), :], ysb[:])

                stages, mk = make_norm_stages(NCK - 1, wide=False)
                stages[0]()  # transpose + reciprocal
                stages[1]()  # transpose back
                # normalize one query chunk at a time; each unlocks 4 F tiles
                for qc in range(NQC):
                    mk([qc])()
                    for tt in range(4 * qc, 4 * qc + 4):
                        emit_f(tt)

            phases.close()

    if not for_sim:
        split_excess_waits(nc)
    return nc


_CACHED = {}


def kernel(x, W_qkv, b_qkv, W_proj, b_proj):
    x = np.asarray(x, dtype=np.float32)
    W_qkv = np.asarray(W_qkv, dtype=np.float32)
    b_qkv = np.asarray(b_qkv, dtype=np.float32)
    W_proj = np.asarray(W_proj, dtype=np.float32)
    b_proj = np.asarray(b_proj, dtype=np.float32)

    zb = not np.any(b_qkv)
    key = ("nc", bool(zb))
    if key not in _CACHED:
        _CACHED[key] = build(zero_bias=bool(zb))
    nc = _CACHED[key]
    _CACHED["nc"] = nc  # convenience handle for external profiling harnesses

    in_maps = []
    for core in range(8):
        b, g = core // 2, core % 2
        cols = np.concatenate(
            [np.arange(i * C + g * CG, i * C + (g + 1) * CG) for i in range(3)]
        )
        in_maps.append(
            {
                "x": np.ascontiguousarray(x[b]),
                "wqkv": np.ascontiguousarray(W_qkv[:, cols]),
                "bqkv": np.ascontiguousarray(b_qkv[cols]),
                "wp": np.ascontiguousarray(W_proj[g * CG : (g + 1) * CG, :]),
            }
        )

    global _LAST_IN_MAPS
    _LAST_IN_MAPS = in_maps
    res = run_bass_kernel_spmd(nc, in_maps, list(range(8))).results
    y = np.empty((B, T, C), dtype=np.float32)
    for b in range(B):
        y[b] = (
            res[2 * b]["yp"].astype(np.float32)
            + res[2 * b + 1]["yp"].astype(np.float32)
            + b_proj[None, :]
        )
    return y
